# revision 8
# baseline (speedup 1.0000x reference)
"""GNN message-passing (graph convolution) kernel for 8 Trainium2 NeuronCores.

    out = relu(segment_sum(h[col], row) + bias),  h = x @ W

Strategy (dst-block sharding — no collectives needed):
  * Host sorts edges by destination node and buckets them into 157 blocks of
    128 dst nodes; blocks are assigned contiguously to cores (20/core).  Each
    core produces a disjoint slice of the output, so partial aggregates never
    need an all-reduce.
  * Phase A (per core, replicated): h = x @ W on the PE in fp16
    (PSUM fp32 accumulate), streamed to a per-core DRAM buffer h[20096,128]
    fp16.  x is shipped pre-transposed/pre-tiled from the host so each lhsT
    tile is one contiguous 64KB DMA.
  * Phase B: for each dst block, dma_gather (SWDGE) fetches the h rows of the
    block's (padded) edge list into SBUF with edge-on-partition layout
    [128e, PB, 128f]; the DVE builds one-hot tiles S[e,n] = (iota == rowloc)
    in fp16; the PE computes out_block += S^T @ val accumulating all chunks of
    the block in PSUM fp32 — an exact segment-sum.  Bias is folded in as an
    extra "bias chunk" per block (gathers a bias row stored at h[20095] with an
    identity one-hot).  ACT applies ReLU PSUM->SBUF, then the result is DMA'd
    out.

Numerics: fp16 operands with fp32 accumulation everywhere; one-hot matmul is
exact, so the only error is fp16 rounding of x, W and h (~1e-3 relative).
"""

import sys

import numpy as np

sys.path.insert(0, "/opt/trn_rl_repo")

import concourse.bacc as bacc  # noqa: E402
import concourse.bass as bass  # noqa: E402  (engine types)
import concourse.mybir as mybir  # noqa: E402
from concourse.bass_utils import run_bass_kernel_spmd  # noqa: E402

N_NODES = 20000
FIN = 256
FOUT = 128
N_EDGES = 640000

NT = 157                 # node tiles of 128 (nodes padded to 20096)
NPAD = NT * 128          # 20096
NBLK = 157               # dst blocks of 128 nodes
NCORES = 8
NB = 20                  # block slots per core (core 7: 17 real + 3 dummy)
BIAS_ROW = NPAD - 1      # h row that phase-B reads the bias vector from

XT_BUFS = 4              # xT tile ring (phase A)
H_BUFS = 4               # h sbuf tile ring (phase A)
S_BUFS = 4               # one-hot tile ring (phase B)

FP16 = mybir.dt.float16
FP32 = mybir.dt.float32
I16 = mybir.dt.int16


def _host_prep(x, edge_index, weight, bias):
    """Cast/retile operands and bucket edges by destination block."""
    x = np.asarray(x, np.float32)
    weight = np.asarray(weight, np.float32)
    bias = np.asarray(bias, np.float32)

    xpad = np.zeros((NPAD, FIN), np.float32)
    xpad[:N_NODES] = x
    # lhsT tiles: xt_tiles[i, k, kc, n] = x[i*128+n, kc*128+k]
    xt_tiles = np.ascontiguousarray(
        xpad.reshape(NT, 128, 2, 128).transpose(0, 3, 2, 1).astype(np.float16)
    )
    w_t = np.ascontiguousarray(weight.astype(np.float16).reshape(2, 128, 128))
    bias16 = np.ascontiguousarray(bias.astype(np.float16).reshape(1, 128))
    iota16 = np.ascontiguousarray(
        np.broadcast_to(np.arange(128, dtype=np.float16), (128, 128))
    )

    row = np.asarray(edge_index[0]).astype(np.int64)
    col = np.asarray(edge_index[1]).astype(np.int64)
    order = np.argsort(row, kind="stable")
    rs = row[order].astype(np.int32)
    cs = col[order].astype(np.int32)

    blk = rs >> 7
    counts = np.bincount(blk, minlength=NBLK)
    starts = np.concatenate([[0], np.cumsum(counts)])
    pb = int(np.max((counts + 127) // 128)) + 1  # +1 for the bias chunk
    pb = ((pb + 6) // 7) * 7  # sub-gathers of 7 chunks (896 idxs <= SWDGE ring)
    nidx = pb * 128
    idxc = nidx // 16

    col16 = np.zeros((NCORES, 128, NB * idxc), np.int16)
    rloc16 = np.full((NCORES, 128, NB * pb), -1.0, np.float32)
    bias_rl = np.arange(128, dtype=np.float32)
    for c in range(NCORES):
        for s in range(NB):
            b = c * NB + s
            lin_col = np.zeros(nidx, np.int32)
            lin_rl = np.full(nidx, -1.0, np.float32)
            lin_col[:128] = BIAS_ROW          # bias chunk: identity one-hot
            lin_rl[:128] = bias_rl
            if b < NBLK:
                e0, e1 = int(starts[b]), int(starts[b + 1])
                k = e1 - e0
                lin_col[128:128 + k] = cs[e0:e1]
                lin_rl[128:128 + k] = rs[e0:e1] - b * 128
            # the SWDGE tx/rx Q7 pair read the indices from different
            # 16-partition groups — replicate the 16-row wrap to all 128
            col16[c, :, s * idxc:(s + 1) * idxc] = np.tile(
                lin_col.reshape(idxc, 16).T.astype(np.int16), (8, 1)
            )
            rloc16[c, :, s * pb:(s + 1) * pb] = (
                lin_rl.reshape(pb, 128).T.astype(np.float32)
            )
    return xt_tiles, w_t, bias16, iota16, col16, rloc16, pb


def _build_program(pb):
    nidx = pb * 128
    idxc = nidx // 16
    nc = bacc.Bacc("TRN2")

    xt_d = nc.dram_tensor("xt", [NT, 128, 2, 128], FP16, kind="ExternalInput")
    w_d = nc.dram_tensor("w", [2, 128, 128], FP16, kind="ExternalInput")
    b_d = nc.dram_tensor("bias", [1, 128], FP16, kind="ExternalInput")
    io_d = nc.dram_tensor("iota", [128, 128], FP16, kind="ExternalInput")
    col_d = nc.dram_tensor("col", [128, NB * idxc], I16, kind="ExternalInput")
    rl_d = nc.dram_tensor("rl", [128, NB * pb], FP32, kind="ExternalInput")
    h_d = nc.dram_tensor("hbuf", [NPAD, 128], FP16)
    o_d = nc.dram_tensor("out", [NB * 128, 128], FP32, kind="ExternalOutput")

    from contextlib import ExitStack

    with ExitStack() as es:
        ph0 = es.enter_context(nc.psum_tensor("ph0", [128, 512], FP32))
        ph1 = es.enter_context(nc.psum_tensor("ph1", [128, 512], FP32))
        ph2 = es.enter_context(nc.psum_tensor("ph2", [128, 512], FP32))
        ph3 = es.enter_context(nc.psum_tensor("ph3", [128, 512], FP32))
        pb0 = es.enter_context(nc.psum_tensor("pb0", [128, 512], FP32))
        pb1 = es.enter_context(nc.psum_tensor("pb1", [128, 512], FP32))
        w_sb = es.enter_context(nc.sbuf_tensor("w_sb", [128, 2, 128], FP16))
        xt_sb = es.enter_context(
            nc.sbuf_tensor("xt_sb", [128, XT_BUFS, 2, 128], FP16)
        )
        h_sb = es.enter_context(nc.sbuf_tensor("h_sb", [128, H_BUFS, 128], FP16))
        iota_sb = es.enter_context(nc.sbuf_tensor("iota_sb", [128, 128], FP16))
        col_sb = es.enter_context(nc.sbuf_tensor("col_sb", [128, NB * idxc], I16))
        rl_sb = es.enter_context(nc.sbuf_tensor("rl_sb", [128, NB * pb], FP32))
        val_sb = es.enter_context(
            nc.sbuf_tensor("val_sb", [128, 2, pb, 128], FP16)
        )
        s_sb = es.enter_context(nc.sbuf_tensor("s_sb", [128, S_BUFS, 128], FP16))
        o_sb = es.enter_context(nc.sbuf_tensor("o_sb", [128, 2, 128], FP32))
        # DMA-completion sems rotate per ring slot (DMA completions on one
        # sem can reorder, so each slot gets its own counter).
        s_ld = [es.enter_context(nc.semaphore(f"s_ld{k}")) for k in range(5)]
        s_xt = [es.enter_context(nc.semaphore(f"s_xt{k}")) for k in range(XT_BUFS)]
        s_hw = [es.enter_context(nc.semaphore(f"s_hw{k}")) for k in range(H_BUFS)]
        s_bw = es.enter_context(nc.semaphore("s_bw"))
        s_gat = [
            es.enter_context(nc.semaphore(f"s_gat{k}"))
            for k in range(2 * (pb // 7))
        ]
        s_ow = [es.enter_context(nc.semaphore(f"s_ow{k}")) for k in range(2)]
        # compute-engine sems increment in program order (no ambiguity)
        s_hmm = es.enter_context(nc.semaphore("s_hmm"))
        s_hcp = es.enter_context(nc.semaphore("s_hcp"))
        s_s = es.enter_context(nc.semaphore("s_s"))
        s_pmm = es.enter_context(nc.semaphore("s_pmm"))
        s_ocp = es.enter_context(nc.semaphore("s_ocp"))
        block = es.enter_context(nc.Block())
        ph = [ph0, ph1, ph2, ph3]
        pbk = [pb0, pb1]

        hw_total = [16 * len(range(k, NT, H_BUFS)) for k in range(H_BUFS)]

        def store_h(sync, j):
            sync.wait_ge(s_hcp, j + 1)
            sync.dma_start(
                h_d[j * 128:(j + 1) * 128, :], h_sb[:, j % H_BUFS, :]
            ).then_inc(s_hw[j % H_BUFS], 16)

        @block.sync
        def _(sync):
            # one-time loads
            sync.dma_start(w_sb[:, 0, :], w_d[0]).then_inc(s_ld[0], 16)
            sync.dma_start(w_sb[:, 1, :], w_d[1]).then_inc(s_ld[1], 16)
            sync.dma_start(iota_sb[:, :], io_d[:, :]).then_inc(s_ld[2], 16)
            sync.dma_start(col_sb[:, :], col_d[:, :]).then_inc(s_ld[3], 16)
            sync.dma_start(rl_sb[:, :], rl_d[:, :]).then_inc(s_ld[4], 16)
            # phase A: stream xT tiles in, h tiles out (staggered)
            for i in range(NT):
                if i >= XT_BUFS:
                    sync.wait_ge(s_hmm, i - (XT_BUFS - 1))
                sync.dma_start(xt_sb[:, i % XT_BUFS, :, :], xt_d[i]).then_inc(
                    s_xt[i % XT_BUFS], 16
                )
                if i >= 3:
                    store_h(sync, i - 3)
            for j in range(NT - 3, NT):
                store_h(sync, j)
            # bias row (after ALL h writes are complete — tile 156 covers it)
            for k in range(H_BUFS):
                sync.wait_ge(s_hw[k], hw_total[k])
            sync.dma_start(h_d[BIAS_ROW:BIAS_ROW + 1, :], b_d[0:1, :]).then_inc(
                s_bw, 16
            )
            # phase B: output stores
            for b in range(NB):
                sync.wait_ge(s_ocp, b + 1)
                sync.dma_start(
                    o_d[b * 128:(b + 1) * 128, :], o_sb[:, b % 2, :]
                ).then_inc(s_ow[b % 2], 16)

        @block.gpsimd
        def _(gpsimd):
            gpsimd.wait_ge(s_ld[3], 16)
            for k in range(H_BUFS):
                gpsimd.wait_ge(s_hw[k], hw_total[k])
            gpsimd.wait_ge(s_bw, 16)
            for b in range(NB):
                if b >= 2:
                    gpsimd.wait_ge(s_pmm, (b - 1) * pb)
                for g in range(pb // 7):
                    gpsimd.dma_gather(
                        val_sb[:, b % 2, g * 7:(g + 1) * 7, :],
                        h_d[:, :],
                        col_sb[:, b * idxc + g * 56:b * idxc + (g + 1) * 56],
                        896,
                        896,
                        128,
                    ).then_inc(s_gat[(b % 2) * (pb // 7) + g], 16)

        @block.tensor
        def _(tensor):
            for k in range(2):
                tensor.wait_ge(s_ld[k], 16)
            # phase A: h tile i = xT_i^T @ W  (two K chunks)
            for i in range(NT):
                tensor.wait_ge(s_xt[i % XT_BUFS], 16 * (i // XT_BUFS + 1))
                if i >= XT_BUFS:
                    tensor.wait_ge(s_hcp, i - (XT_BUFS - 1))
                tensor.matmul(
                    ph[i % XT_BUFS][:, 0:128],
                    xt_sb[:, i % XT_BUFS, 0, :],
                    w_sb[:, 0, :],
                    start=True,
                    stop=False,
                )
                tensor.matmul(
                    ph[i % XT_BUFS][:, 0:128],
                    xt_sb[:, i % XT_BUFS, 1, :],
                    w_sb[:, 1, :],
                    start=False,
                    stop=True,
                ).then_inc(s_hmm, 1)
            # phase B: out_block += S_chunk^T @ val_chunk
            for b in range(NB):
                if b >= 2:
                    tensor.wait_ge(s_ocp, b - 1)
                for c in range(pb):
                    j = b * pb + c
                    if c % 7 == 0:
                        tensor.wait_ge(
                            s_gat[(b % 2) * (pb // 7) + c // 7],
                            16 * (b // 2 + 1),
                        )
                    tensor.wait_ge(s_s, j + 1)
                    tensor.matmul(
                        pbk[b % 2][:, 0:128],
                        s_sb[:, j % S_BUFS, :],
                        val_sb[:, b % 2, c, :],
                        start=(c == 0),
                        stop=(c == pb - 1),
                    ).then_inc(s_pmm, 1)

        @block.vector
        def _(vector):
            # phase A: PSUM fp32 -> SBUF fp16
            for i in range(NT):
                vector.wait_ge(s_hmm, i + 1)
                if i >= H_BUFS:
                    vector.wait_ge(s_hw[i % H_BUFS], 16 * (i // H_BUFS))
                vector.tensor_copy(
                    h_sb[:, i % H_BUFS, :], ph[i % XT_BUFS][:, 0:128]
                ).then_inc(s_hcp, 1)
            # phase B: one-hot tiles S[e, n] = (iota[n] == rowloc[e])
            vector.wait_ge(s_ld[2], 16)
            vector.wait_ge(s_ld[4], 16)
            for j in range(NB * pb):
                if j >= S_BUFS:
                    vector.wait_ge(s_pmm, j - (S_BUFS - 1))
                vector.tensor_scalar(
                    s_sb[:, j % S_BUFS, :],
                    iota_sb[:, :],
                    rl_sb[:, j:j + 1],
                    None,
                    mybir.AluOpType.is_equal,
                ).then_inc(s_s, 1)

        @block.scalar
        def _(scalar):
            for b in range(NB):
                scalar.wait_ge(s_pmm, (b + 1) * pb)
                if b >= 2:
                    scalar.wait_ge(s_ow[b % 2], 16 * (b // 2))
                scalar.activation(
                    o_sb[:, b % 2, :],
                    pbk[b % 2][:, 0:128],
                    mybir.ActivationFunctionType.Relu,
                ).then_inc(s_ocp, 1)

    nc.compile()
    return nc


def _run(x, edge_index, weight, bias, trace=False):
    xt_tiles, w_t, bias16, iota16, col16, rloc16, pb = _host_prep(
        x, edge_index, weight, bias
    )
    nc = _build_program(pb)
    in_maps = [
        {
            "xt": xt_tiles,
            "w": w_t,
            "bias": bias16,
            "iota": iota16,
            "col": np.ascontiguousarray(col16[c]),
            "rl": np.ascontiguousarray(rloc16[c]),
        }
        for c in range(NCORES)
    ]
    res = run_bass_kernel_spmd(nc, in_maps, list(range(NCORES)), trace=trace)
    out = np.concatenate([res.results[c]["out"] for c in range(NCORES)], axis=0)
    return np.ascontiguousarray(out[:N_NODES]), res


def kernel(x, edge_index, weight, bias):
    out, _ = _run(x, edge_index, weight, bias, trace=False)
    return out
